# revision 10
# baseline (speedup 1.0000x reference)
"""Trainium2 Bass kernel for nn_LocalDenseCrossReadout (v4, folded projections).

Strategy:
- Data-parallel over batch: 8 batches -> 8 NeuronCores, one batch per core.
- Host-side algebraic folds (the big win vs v3):
  * k-projection eliminated: scores = q_p @ k_p^T = (q_p @ Wk^T) @ xn_s^T
    (+ per-row constant q_p.rk which cancels in softmax), so we fold
    Wqk = Wq_f @ Wk^T per batch and project only the q side (1024 rows
    instead of 4096).
  * v-projection and output projection merged: attn @ (xn_s @ Wv) @ Wo =
    (attn @ xn_s) @ (Wv @ Wo), so attention reads raw LN'd source rows and
    the output projection uses Wvo = Wv @ Wo. v bias folds into the output
    bias (softmax rows sum to 1).
  * gate projections: gate_q rides as extra columns of the q projection;
    gate_k = xn_s @ (Wk @ Wgk) is a small fp8 DoubleRow projection.
  * band mask is added to the GATE logits (tanh(-1e30) = -1 makes
    P = (1+t)*e vanish), removing it from the score psum chain.
- Device kernel per core: LN (bn_stats/aggr, Rsqrt, fused (x-mu)*rstd on
  Pool), PE transposes into bf16 (q) / fp8 (s) column-major copies, bf16
  q-projection, fp8 DoubleRow scores and gate-k, banded attention per
  128-row q tile software-pipelined, attn@xn via DMA-transposed P, output
  projection with normalize+bias fused into one DVE op.
"""

import sys

sys.path.insert(0, "/opt/trn_rl_repo")

import numpy as np

import concourse.bass as bass
import concourse.tile as tile
from concourse import bacc
from concourse import mybir
from concourse.bass_utils import run_bass_kernel_spmd
from concourse.masks import make_identity

DIM, QS, QT, KS, KT, WIN, B, RANK = 512, 64, 16, 256, 16, 4, 8, 32
Q = QS * QT  # 1024
K = KS * KT  # 4096
WINW = 768  # aligned kv window per 128-row q tile
NQT = Q // 128  # 8 q tiles
NKV = K // 128  # 32 kv tiles
F32 = mybir.dt.float32
BF16 = mybir.dt.bfloat16
F8 = mybir.dt.float8e4
FT = mybir.ActivationFunctionType
ALU = mybir.AluOpType

QK_PRESC = 64.0   # q-proj psum = QK_PRESC * scale * qk ; exp descales
GK_PRESC = 256.0  # gate-k psum prescale (fp8 weight range)

# kv window start (aligned to 128) per q tile
WSTARTS = [0, 384, 896, 1408, 1920, 2432, 2944, 3328]
# last s-bank (512 rows each) needed per q tile
NEED_SBANK = [(w + WINW + 511) // 512 - 1 for w in WSTARTS]


def build_bass():
    nc = bacc.Bacc("TRN2", target_bir_lowering=False)
    q = nc.dram_tensor("q", [Q, DIM], BF16, kind="ExternalInput")
    s = nc.dram_tensor("s", [K, DIM], BF16, kind="ExternalInput")
    wqk = nc.dram_tensor("wqk", [DIM, DIM + RANK], BF16, kind="ExternalInput")
    wkg = nc.dram_tensor("wkg", [DIM, RANK], F8, kind="ExternalInput")
    wvo = nc.dram_tensor("wvo", [DIM, DIM], BF16, kind="ExternalInput")
    rqt = nc.dram_tensor("rqt", [128, 6], F32, kind="ExternalInput")
    bo2r = nc.dram_tensor("bo2r", [128, DIM], F32, kind="ExternalInput")
    bmask = nc.dram_tensor("bmask", [NQT, 128, WINW], BF16, kind="ExternalInput")
    out = nc.dram_tensor("out", [Q, DIM], F32, kind="ExternalOutput")

    with tile.TileContext(nc) as tc:
        with (
            tc.tile_pool(name="consts", bufs=1) as consts,
            tc.tile_pool(name="wts", bufs=1) as wts,
            tc.tile_pool(name="big", bufs=1) as big,
            tc.tile_pool(name="xin", bufs=3) as xin,
            tc.tile_pool(name="xnp", bufs=4) as xnp,
            tc.tile_pool(name="stats", bufs=8) as stats,
            tc.tile_pool(name="attn", bufs=3) as attn,
            tc.tile_pool(name="msks", bufs=1) as msks,
            tc.tile_pool(name="ps_t", bufs=2, space="PSUM") as ps_t,
            tc.tile_pool(name="ps_p", bufs=2, space="PSUM") as ps_p,
            tc.tile_pool(name="ps_b", bufs=2, space="PSUM") as ps_b,
        ):
            # ---------------- constants ----------------
            identb = consts.tile([128, 128], BF16)
            make_identity(nc, identb)
            idn64 = consts.tile([128, 128], BF16)
            make_identity(nc, idn64)
            nc.gpsimd.tensor_scalar_mul(idn64, idn64, -float(QK_PRESC))
            eps = consts.tile([128, 1], F32)
            nc.vector.memset(eps, 1e-5)

            # persistent activations
            qt_big = big.tile([128, 4, Q], BF16, tag="qt_big")   # xn_q^T (bf16)
            st8 = big.tile([128, 4, K], F8, tag="st8")           # xn_s^T (fp8)
            xnr = big.tile([128, NKV, DIM], BF16, tag="xnr")     # xn_s rows
            qkT = big.tile([128, 4, Q], F8, tag="qkT")           # (qk*scale*64)^T
            gq = big.tile([32, Q], BF16, tag="gq")               # gate_q^T
            gk = big.tile([32, K], BF16, tag="gk")               # gate_k^T

            # ---- stage 1: load one bank (512 rows) in one DMA
            def ln_load(src_dram, row0):
                x4 = xin.tile([128, 4, DIM], BF16, tag="x")
                nc.sync.dma_start(
                    out=x4, in_=src_dram[row0:row0 + 512, :].rearrange(
                        "(j p) n -> p j n", p=128))
                return x4

            # ---- stage 2a: LN stats for one bank (4 tiles), batched rstd
            def ln_stats_bank(x4):
                mv4 = stats.tile([128, 4, 2], F32, tag="mv4")
                for j in range(4):
                    st6 = stats.tile([128, 6], F32, tag="st6")
                    nc.vector.bn_stats(out=st6, in_=x4[:, j, :])
                    nc.vector.bn_aggr(out=mv4[:, j, :], in_=st6)
                lv4 = stats.tile([128, 4], F32, tag="lv4")
                nc.scalar.activation(out=lv4, in_=mv4[:, :, 1],
                                     func=FT.Ln, bias=eps, scale=1.0)
                rstd4 = stats.tile([128, 4], F32, tag="rstd4")
                nc.scalar.activation(out=rstd4, in_=lv4,
                                     func=FT.Exp, bias=0.0, scale=-0.5)
                nmr4 = stats.tile([128, 4], F32, tag="nmr4")
                nc.vector.scalar_tensor_tensor(
                    out=nmr4, in0=mv4[:, :, 0], scalar=-1.0, in1=rstd4,
                    op0=ALU.mult, op1=ALU.mult)
                return nmr4, rstd4

            # ---- stage 2b: LN apply + PE transpose (2 tiles / psum buf)
            def ln_tile(x, xn_dst, pt2, joff, nmr4, rstd4, j):
                nc.gpsimd.tensor_scalar(out=xn_dst, in0=x,
                                        scalar1=rstd4[:, j:j + 1],
                                        scalar2=nmr4[:, j:j + 1],
                                        op0=ALU.mult, op1=ALU.add)
                for c in range(4):
                    nc.tensor.transpose(pt2[:, c, joff:joff + 128],
                                        xn_dst[:, c * 128:(c + 1) * 128], identb)

            # ---- q projection (qk + gate_q) for one q bank of 512 rows
            def proj_q_bank(col0):
                for m in range(5):
                    mw = 128 if m < 4 else RANK
                    mo = m * 128
                    pp = ps_p.tile([128, DIM], F32, tag="ps")
                    for c in range(4):
                        nc.tensor.matmul(pp[:mw, :], wqk_sb[:, c, mo:mo + mw],
                                         qt_big[:, c, col0:col0 + 512],
                                         start=(c == 0), stop=(c == 3))
                    if m < 4:
                        nc.scalar.activation(out=qkT[:, m, col0:col0 + 512],
                                             in_=pp, func=FT.Identity,
                                             bias=rqt_sb[:, m:m + 1], scale=1.0)
                    else:
                        nc.scalar.activation(out=gq[:, col0:col0 + 512],
                                             in_=pp[:RANK, :], func=FT.Identity,
                                             bias=rqt_sb[:RANK, 4:5], scale=1.0)

            # ---- gate_k projection (fp8 DoubleRow) for one s bank
            def proj_gk_bank(col0):
                pp = ps_p.tile([128, DIM], F32, tag="ps")
                for c in (0, 2):
                    nc.tensor.matmul(pp[:RANK, :], wkg_sb[:, c:c + 2, :],
                                     st8[:, c:c + 2, col0:col0 + 512],
                                     start=(c == 0), stop=(c == 2),
                                     perf_mode=mybir.MatmulPerfMode.DoubleRow)
                nc.scalar.activation(out=gk[:, col0:col0 + 512],
                                     in_=pp[:RANK, :], func=FT.Identity,
                                     bias=rqt_sb[:RANK, 5:6],
                                     scale=1.0 / GK_PRESC)

            state = {}

            def attn_front(t):
                """gate logits (+mask), tanh, scores, exp, P, P^T."""
                w0 = WSTARTS[t]
                qc = bass.ts(t, 128)
                gl = ps_b.tile([128, WINW], F32, tag="glsc")
                for n0, nn_ in ((0, 512), (512, 256)):
                    nc.tensor.matmul(gl[:, n0:n0 + nn_], gq[:, qc],
                                     gk[:, w0 + n0:w0 + n0 + nn_],
                                     start=True, stop=True)
                # sigmoid(gl)*exp(score) = exp(score - ln(1+exp(-gl)));
                # the dropped factor 2 cancels in softmax
                u = attn.tile([128, WINW], BF16, tag="u")
                nc.scalar.activation(out=u, in_=gl, func=FT.Exp,
                                     bias=0.0, scale=-1.0)
                w = attn.tile([128, WINW], BF16, tag="w")
                nc.scalar.activation(out=w, in_=u, func=FT.Ln,
                                     bias=1.0, scale=1.0)
                sc = ps_b.tile([128, WINW], F32, tag="glsc")
                for n0, nn_ in ((0, 512), (512, 256)):
                    for c in (0, 2):
                        nc.tensor.matmul(sc[:, n0:n0 + nn_],
                                         qkT[:, c:c + 2, qc],
                                         st8[:, c:c + 2, w0 + n0:w0 + n0 + nn_],
                                         start=(c == 0), stop=False,
                                         perf_mode=mybir.MatmulPerfMode.DoubleRow)
                    nc.tensor.matmul(sc[:, n0:n0 + nn_], idn64,
                                     w[:, n0:n0 + nn_], start=False, stop=False)
                    nc.tensor.matmul(sc[:, n0:n0 + nn_], identb,
                                     msk_t[t][:, n0:n0 + nn_],
                                     start=False, stop=True)
                P = attn.tile([128, WINW], BF16, tag="P")
                rsum = stats.tile([128, 1], F32, tag="rsum")
                nc.scalar.activation(out=P, in_=sc, func=FT.Exp, bias=0.0,
                                     scale=1.0 / QK_PRESC, accum_out=rsum)
                rinv = stats.tile([128, 1], F32, tag="rinv")
                nc.vector.reciprocal(out=rinv, in_=rsum)
                aT = attn.tile([128, 6, 128], BF16, tag="aT")
                nc.sync.dma_start_transpose(aT, P)
                state[t] = (aT, rinv)

            def attn_mid(t):
                """attn@xn (unnormalized), oa^T via DMA transpose."""
                w0 = WSTARTS[t]
                aT, rinv = state.pop(t)
                av = ps_p.tile([128, DIM], F32, tag="ps")
                for cc in range(6):
                    nc.tensor.matmul(av, aT[:, cc, :], xnr[:, w0 // 128 + cc, :],
                                     start=(cc == 0), stop=(cc == 5))
                oa = attn.tile([128, DIM], BF16, tag="oa")
                nc.vector.tensor_copy(oa, av)
                oaT = attn.tile([128, 4, 128], BF16, tag="oaT")
                nc.sync.dma_start_transpose(oaT, oa)
                state[(t, "fin")] = (oaT, rinv)

            def attn_fin(t):
                """output projection, fused normalize + bias, store."""
                qc = bass.ts(t, 128)
                oaT, rinv = state.pop((t, "fin"))
                fin = ps_p.tile([128, DIM], F32, tag="ps")
                for c in range(4):
                    nc.tensor.matmul(fin, oaT[:, c, :], wvo_sb[:, c, :],
                                     start=(c == 0), stop=(c == 3))
                ob = attn.tile([128, DIM], F32, tag="ob")
                nc.vector.scalar_tensor_tensor(
                    out=ob, in0=fin, scalar=rinv, in1=bo2r_sb,
                    op0=ALU.mult, op1=ALU.add)
                nc.gpsimd.dma_start(out=out[qc, :], in_=ob)

            # ---- bank schedule: q0, s0, s1, q1, s2..s7 -----------------
            BANKS = [("q", 0), ("s", 0), ("s", 1), ("q", 1),
                     ("s", 2), ("s", 3), ("s", 4), ("s", 5), ("s", 6), ("s", 7)]

            def bank_src(bk):
                kind, idx = BANKS[bk]
                return (q, idx * 512) if kind == "q" else (s, idx * 512)

            prog = {"f": 0, "m": 0, "o": 0}

            def pump(done_sbank):
                while prog["f"] < NQT and NEED_SBANK[prog["f"]] <= done_sbank:
                    attn_front(prog["f"])
                    prog["f"] += 1
                    while prog["m"] < max(0, prog["f"] - 1):
                        attn_mid(prog["m"])
                        prog["m"] += 1
                    while prog["o"] < max(0, prog["m"] - 1):
                        attn_fin(prog["o"])
                        prog["o"] += 1

            # input bank 0 first on the DMA queue so LN starts immediately;
            # then small weights (needed early), then the rest behind.
            pending = [ln_load(*bank_src(0))]
            rqt_sb = consts.tile([128, 6], F32)
            nc.sync.dma_start(out=rqt_sb, in_=rqt[:, :])
            wqk_sb = wts.tile([128, 4, DIM + RANK], BF16, tag="wqk")
            nc.sync.dma_start(
                out=wqk_sb, in_=wqk[:, :].rearrange("(c p) n -> p c n", p=128))
            wkg_sb = wts.tile([128, 4, RANK], F8, tag="wkg")
            nc.sync.dma_start(
                out=wkg_sb, in_=wkg[:, :].rearrange("(c p) n -> p c n", p=128))
            pending.append(ln_load(*bank_src(1)))
            mskall = msks.tile([128, NQT, WINW], BF16, tag="msk")
            nc.sync.dma_start(
                out=mskall,
                in_=bmask[:, :, :].rearrange("t p n -> p t n"))
            msk_t = [mskall[:, t, :] for t in range(NQT)]
            wvo_sb = wts.tile([128, 4, DIM], BF16, tag="wvo")
            nc.sync.dma_start(
                out=wvo_sb, in_=wvo[:, :].rearrange("(c p) n -> p c n", p=128))
            bo2r_sb = consts.tile([128, DIM], F32)
            nc.sync.dma_start(out=bo2r_sb, in_=bo2r[:, :])

            for bk in range(10):
                kind, idx = BANKS[bk]
                xq = pending.pop(0)
                if bk + 2 < 10:
                    pending.append(ln_load(*bank_src(bk + 2)))
                col0 = idx * 512
                nmr4, rstd4 = ln_stats_bank(xq)
                dst_big = qt_big if kind == "q" else st8
                for jj in range(2):
                    pt2 = ps_t.tile([128, 4, 256], BF16, tag="pt")
                    for j2 in range(2):
                        j = jj * 2 + j2
                        if kind == "q":
                            xn = xnp.tile([128, DIM], BF16, tag="xn")
                        else:
                            xn = xnr[:, idx * 4 + j, :]
                        ln_tile(xq[:, j, :], xn, pt2, j2 * 128, nmr4, rstd4, j)
                    if jj == 0:
                        nc.vector.tensor_copy(
                            dst_big[:, :, col0:col0 + 256], pt2)
                    else:
                        nc.scalar.activation(
                            out=dst_big[:, :, col0 + 256:col0 + 512],
                            in_=pt2, func=FT.Identity, bias=0.0, scale=1.0)
                if kind == "q":
                    proj_q_bank(col0)
                else:
                    proj_gk_bank(col0)
                    pump(idx)
            while prog["m"] < NQT:
                attn_mid(prog["m"])
                prog["m"] += 1
            while prog["o"] < NQT:
                attn_fin(prog["o"])
                prog["o"] += 1

    if not nc.is_finalized():
        nc.finalize()
    return nc


_NC_CACHE = None


def _get_nc():
    global _NC_CACHE
    if _NC_CACHE is None:
        _NC_CACHE = build_bass()
    return _NC_CACHE


def _host_fold(inputs):
    f32 = np.float32
    bf16 = mybir.dt.np(BF16)
    f8 = mybir.dt.np(F8)
    scale = f32(DIM ** -0.5)
    sqr = f32(np.sqrt(RANK))
    ctx0 = np.asarray(inputs["ctx0"], f32)
    ctx1 = np.asarray(inputs["ctx1"], f32)
    pre = ctx0 @ inputs["Wc0"] + inputs["bc0"] + ctx1 @ inputs["Wc1"] + inputs["bc1"]
    pre = np.asarray(pre, f32)
    h = pre / (1.0 + np.exp(-pre))
    gbv = np.asarray(h @ inputs["Wf"] + inputs["bf"], f32)
    gamma, beta = gbv[:, :DIM], gbv[:, DIM:]

    qn_g = np.asarray(inputs["qn_g"], f32)
    qn_b = np.asarray(inputs["qn_b"], f32)
    kvn_g = np.asarray(inputs["kvn_g"], f32)
    kvn_b = np.asarray(inputs["kvn_b"], f32)
    Wq, bq = np.asarray(inputs["Wq"], f32), np.asarray(inputs["bq"], f32)
    Wk, bk = np.asarray(inputs["Wk"], f32), np.asarray(inputs["bk"], f32)
    Wv, bv = np.asarray(inputs["Wv"], f32), np.asarray(inputs["bv"], f32)
    Wo, bo = np.asarray(inputs["Wo"], f32), np.asarray(inputs["bo"], f32)
    Wgq = np.asarray(inputs["Wgq"], f32)
    Wgk = np.asarray(inputs["Wgk"], f32)
    mask = np.asarray(inputs["mask"], f32)

    # batch-independent folds
    WkS = Wk * kvn_g[:, None]
    rk = (kvn_b @ Wk + bk).astype(f32)
    WvS = Wv * kvn_g[:, None]
    rv = (kvn_b @ Wv + bv).astype(f32)
    wvo = np.ascontiguousarray(WvS @ Wo).astype(bf16)          # [512, 512]
    bo2r = np.broadcast_to((rv @ Wo + bo).astype(f32), (128, DIM))
    bo2r = np.ascontiguousarray(bo2r)
    wkg8 = np.ascontiguousarray((WkS @ Wgk) * GK_PRESC).astype(f8)
    rkg = (rk @ Wgk).astype(f32)                               # true gate-k bias

    bmask = np.stack([mask[t * 128:(t + 1) * 128, w:w + WINW]
                      for t, w in enumerate(WSTARTS)])
    bmask = (np.maximum(bmask, -50.0) * QK_PRESC).astype(bf16)

    query = np.asarray(inputs["query"], f32).reshape(B, Q, DIM)
    source = np.asarray(inputs["source"], f32).reshape(B, K, DIM)

    in_maps = []
    for b in range(B):
        sg = qn_g * (1.0 + gamma[b])
        off = qn_b * (1.0 + gamma[b]) + beta[b]
        Wq_f = Wq * sg[:, None]
        rq_raw = (off @ Wq + bq).astype(f32)
        Wqk = (Wq_f @ WkS.T) * (scale * QK_PRESC)
        rqk = (rq_raw @ WkS.T) * (scale * QK_PRESC)
        wgq_f = (Wq_f @ Wgq) / sqr
        rgq = rq_raw @ Wgq / sqr
        wqk_ext = np.concatenate([Wqk, wgq_f], axis=1)
        rqt_b = np.zeros((128, 6), f32)
        rqt_b[:, :4] = rqk.reshape(4, 128).T
        rqt_b[:RANK, 4] = rgq
        rqt_b[:RANK, 5] = rkg
        in_maps.append({
            "q": query[b].astype(bf16),
            "s": source[b].astype(bf16),
            "wqk": wqk_ext.astype(bf16),
            "wkg": wkg8,
            "wvo": wvo,
            "rqt": rqt_b,
            "bo2r": bo2r,
            "bmask": bmask,
        })
    return in_maps


def kernel(**inputs):
    nc = _get_nc()
    in_maps = _host_fold(inputs)
    res = run_bass_kernel_spmd(nc, in_maps, core_ids=list(range(B)))
    out = np.stack([res.results[b]["out"] for b in range(B)])
    return out.reshape(B, QS, QT, DIM).astype(np.float32)


if __name__ == "__main__":
    build_bass()
    print("bass build OK")


# revision 11
# speedup vs baseline: 1.1207x; 1.1207x over previous
"""Trainium2 Bass kernel for nn_LocalDenseCrossReadout (v5, folded projections).

Strategy:
- Data-parallel over batch: 8 batches -> 8 NeuronCores, one batch per core.
- Host-side algebraic folds:
  * k-projection eliminated: scores = q_p @ k_p^T = (q_p @ Wk^T) @ xn_s^T
    (+ per-q-row constant q_p.rk which cancels in softmax), so we fold
    Wqk = Wq_f @ Wk^T per batch and project only the 1024 q rows.
  * v-projection and output projection merged: attn @ (xn_s @ Wv) @ Wo =
    (attn @ xn_s) @ (Wv @ Wo): attention reads raw LN'd source rows and the
    output projection uses Wvo = Wv @ Wo; v bias folds into the output bias.
  * gate_q rides as extra columns of the q projection; gate_k =
    xn_s @ (Wk @ Wgk) is a small fp8 DoubleRow projection.
  * LayerNorm statistics (row mean/rstd) are computed on host in f32 and
    shipped as a [128, 40, 2] tensor; the device applies x*rstd + (-mu*rstd)
    on Pool. This keeps the Scalar activation table set to {exp, tanh,
    identity} (one table, no reload thrash).
- Device per core: LN apply + PE transposes (bf16 q / fp8 s column-major),
  bf16 q-projection, fp8 DoubleRow gate-k + scores, banded attention per
  128-row q tile (tanh gate, mask folded into score psum, P=(1+t)*e with
  fused row-sum), attn@xn via DMA-transposed P, output projection with
  normalize+bias fused into one DVE op.
"""

import sys

sys.path.insert(0, "/opt/trn_rl_repo")

import numpy as np

import concourse.bass as bass
import concourse.tile as tile
from concourse import bacc
from concourse import mybir
from concourse.bass_utils import run_bass_kernel_spmd
from concourse.masks import make_identity

DIM, QS, QT, KS, KT, WIN, B, RANK = 512, 64, 16, 256, 16, 4, 8, 32
Q = QS * QT  # 1024
K = KS * KT  # 4096
WINW = 768  # aligned kv window per 128-row q tile
NQT = Q // 128  # 8 q tiles
NKV = K // 128  # 32 kv tiles
NT = NQT + NKV  # 40 LN tiles
F32 = mybir.dt.float32
BF16 = mybir.dt.bfloat16
F8 = mybir.dt.float8e4
FT = mybir.ActivationFunctionType
ALU = mybir.AluOpType

QK_PRESC = 64.0   # q-proj psum = QK_PRESC * scale * qk ; exp descales
GK_PRESC = 256.0  # gate-k psum prescale (fp8 weight range)

# kv window start (aligned to 128) per q tile
WSTARTS = [0, 384, 896, 1408, 1920, 2432, 2944, 3328]
# last s-bank (512 rows each) needed per q tile
NEED_SBANK = [(w + WINW + 511) // 512 - 1 for w in WSTARTS]


def build_bass():
    nc = bacc.Bacc("TRN2", target_bir_lowering=False)
    q = nc.dram_tensor("q", [Q, DIM], BF16, kind="ExternalInput")
    s = nc.dram_tensor("s", [K, DIM], BF16, kind="ExternalInput")
    wqk = nc.dram_tensor("wqk", [DIM, DIM + RANK], BF16, kind="ExternalInput")
    wkg = nc.dram_tensor("wkg", [DIM, RANK], F8, kind="ExternalInput")
    wvo = nc.dram_tensor("wvo", [DIM, DIM], BF16, kind="ExternalInput")
    rqt = nc.dram_tensor("rqt", [128, 6], F32, kind="ExternalInput")
    lns = nc.dram_tensor("lns", [128, NT, 2], F32, kind="ExternalInput")
    bo2r = nc.dram_tensor("bo2r", [128, DIM], F32, kind="ExternalInput")
    bmask = nc.dram_tensor("bmask", [NQT, 128, WINW], BF16, kind="ExternalInput")
    out = nc.dram_tensor("out", [Q, DIM], F32, kind="ExternalOutput")

    with tile.TileContext(nc) as tc:
        with (
            tc.tile_pool(name="consts", bufs=1) as consts,
            tc.tile_pool(name="wts", bufs=1) as wts,
            tc.tile_pool(name="big", bufs=1) as big,
            tc.tile_pool(name="xin", bufs=3) as xin,
            tc.tile_pool(name="xnp", bufs=4) as xnp,
            tc.tile_pool(name="stats", bufs=8) as stats,
            tc.tile_pool(name="attn", bufs=3) as attn,
            tc.tile_pool(name="msks", bufs=1) as msks,
            tc.tile_pool(name="ps_t", bufs=2, space="PSUM") as ps_t,
            tc.tile_pool(name="ps_p", bufs=2, space="PSUM") as ps_p,
            tc.tile_pool(name="ps_b", bufs=2, space="PSUM") as ps_b,
        ):
            # ---------------- constants ----------------
            identb = consts.tile([128, 128], BF16)
            make_identity(nc, identb)

            # persistent activations
            qt_big = big.tile([128, 4, Q], BF16, tag="qt_big")   # xn_q^T (bf16)
            st8 = big.tile([128, 4, K], F8, tag="st8")           # xn_s^T (fp8)
            xnr = big.tile([128, NKV, DIM], BF16, tag="xnr")     # xn_s rows
            qkT = big.tile([128, 4, Q], F8, tag="qkT")           # (qk*scale*64)^T
            gq = big.tile([32, Q], BF16, tag="gq")               # gate_q^T
            gk = big.tile([32, K], BF16, tag="gk")               # gate_k^T

            # ---- stage 1: load one bank (512 rows) in one DMA
            def ln_load(src_dram, row0):
                x4 = xin.tile([128, 4, DIM], BF16, tag="x")
                nc.sync.dma_start(
                    out=x4, in_=src_dram[row0:row0 + 512, :].rearrange(
                        "(j p) n -> p j n", p=128))
                return x4

            # ---- stage 2: LN apply (host stats) + PE transpose
            def ln_tile(x, xn_dst, pt2, joff, g):
                nc.gpsimd.tensor_scalar(out=xn_dst, in0=x,
                                        scalar1=lns_sb[:, g, 0:1],
                                        scalar2=lns_sb[:, g, 1:2],
                                        op0=ALU.mult, op1=ALU.add)
                for c in range(4):
                    nc.tensor.transpose(pt2[:, c, joff:joff + 128],
                                        xn_dst[:, c * 128:(c + 1) * 128], identb)

            # ---- q projection (qk + gate_q) for one q bank of 512 rows
            def proj_q_bank(col0):
                for m in range(5):
                    mw = 128 if m < 4 else RANK
                    mo = m * 128
                    pp = ps_p.tile([128, DIM], F32, tag="ps")
                    for c in range(4):
                        nc.tensor.matmul(pp[:mw, :], wqk_sb[:, c, mo:mo + mw],
                                         qt_big[:, c, col0:col0 + 512],
                                         start=(c == 0), stop=(c == 3))
                    if m < 4:
                        nc.scalar.activation(out=qkT[:, m, col0:col0 + 512],
                                             in_=pp, func=FT.Identity,
                                             bias=rqt_sb[:, m:m + 1], scale=1.0)
                    else:
                        nc.scalar.activation(out=gq[:, col0:col0 + 512],
                                             in_=pp[:RANK, :], func=FT.Identity,
                                             bias=rqt_sb[:RANK, 4:5], scale=1.0)

            # ---- gate_k projection (fp8 DoubleRow) for one s bank
            def proj_gk_bank(col0):
                pp = ps_p.tile([128, DIM], F32, tag="ps")
                for c in (0, 2):
                    nc.tensor.matmul(pp[:RANK, :], wkg_sb[:, c:c + 2, :],
                                     st8[:, c:c + 2, col0:col0 + 512],
                                     start=(c == 0), stop=(c == 2),
                                     perf_mode=mybir.MatmulPerfMode.DoubleRow)
                nc.scalar.activation(out=gk[:, col0:col0 + 512],
                                     in_=pp[:RANK, :], func=FT.Identity,
                                     bias=rqt_sb[:RANK, 5:6],
                                     scale=1.0 / GK_PRESC)

            state = {}

            def attn_front(t):
                """gate logits, tanh, scores+mask, exp, P, P^T."""
                w0 = WSTARTS[t]
                qc = bass.ts(t, 128)
                gl = ps_b.tile([128, WINW], F32, tag="glsc")
                for n0, nn_ in ((0, 512), (512, 256)):
                    nc.tensor.matmul(gl[:, n0:n0 + nn_], gq[:, qc],
                                     gk[:, w0 + n0:w0 + n0 + nn_],
                                     start=True, stop=True)
                # 2*sigmoid(gl) = 1 + tanh(gl/2); factor 2 cancels in softmax
                tq = attn.tile([128, WINW], BF16, tag="tq")
                nc.scalar.activation(out=tq, in_=gl, func=FT.Tanh,
                                     bias=0.0, scale=0.5)
                sc = ps_b.tile([128, WINW], F32, tag="glsc")
                for n0, nn_ in ((0, 512), (512, 256)):
                    for c in (0, 2):
                        nc.tensor.matmul(sc[:, n0:n0 + nn_],
                                         qkT[:, c:c + 2, qc],
                                         st8[:, c:c + 2, w0 + n0:w0 + n0 + nn_],
                                         start=(c == 0), stop=False,
                                         perf_mode=mybir.MatmulPerfMode.DoubleRow)
                    nc.tensor.matmul(sc[:, n0:n0 + nn_], identb,
                                     msk_t[t][:, n0:n0 + nn_],
                                     start=False, stop=True)
                e = attn.tile([128, WINW], BF16, tag="e")
                nc.scalar.activation(out=e, in_=sc, func=FT.Exp, bias=0.0,
                                     scale=1.0 / QK_PRESC)
                P = attn.tile([128, WINW], BF16, tag="P")
                rsum = stats.tile([128, 1], F32, tag="rsum")
                nc.vector.scalar_tensor_tensor(
                    out=P, in0=tq, scalar=1.0, in1=e,
                    op0=ALU.add, op1=ALU.mult, accum_out=rsum)
                rinv = stats.tile([128, 1], F32, tag="rinv")
                nc.vector.reciprocal(out=rinv, in_=rsum)
                aT = attn.tile([128, 6, 128], BF16, tag="aT")
                nc.sync.dma_start_transpose(aT, P)
                state[t] = (aT, rinv)

            def attn_mid(t):
                """attn@xn (unnormalized), oa^T via DMA transpose."""
                w0 = WSTARTS[t]
                aT, rinv = state.pop(t)
                av = ps_p.tile([128, DIM], F32, tag="ps")
                for cc in range(6):
                    nc.tensor.matmul(av, aT[:, cc, :], xnr[:, w0 // 128 + cc, :],
                                     start=(cc == 0), stop=(cc == 5))
                oa = attn.tile([128, DIM], BF16, tag="oa")
                nc.vector.tensor_copy(oa, av)
                oaT = attn.tile([128, 4, 128], BF16, tag="oaT")
                nc.sync.dma_start_transpose(oaT, oa)
                state[(t, "fin")] = (oaT, rinv)

            def attn_fin(t):
                """output projection, fused normalize + bias, store."""
                qc = bass.ts(t, 128)
                oaT, rinv = state.pop((t, "fin"))
                fin = ps_p.tile([128, DIM], F32, tag="ps")
                for c in range(4):
                    nc.tensor.matmul(fin, oaT[:, c, :], wvo_sb[:, c, :],
                                     start=(c == 0), stop=(c == 3))
                ob = attn.tile([128, DIM], F32, tag="ob")
                nc.vector.scalar_tensor_tensor(
                    out=ob, in0=fin, scalar=rinv, in1=bo2r_sb,
                    op0=ALU.mult, op1=ALU.add)
                nc.gpsimd.dma_start(out=out[qc, :], in_=ob)

            # ---- bank schedule: q0, s0, s1, q1, s2..s7 -----------------
            BANKS = [("q", 0), ("s", 0), ("s", 1), ("q", 1),
                     ("s", 2), ("s", 3), ("s", 4), ("s", 5), ("s", 6), ("s", 7)]

            def bank_src(bk):
                kind, idx = BANKS[bk]
                return (q, idx * 512) if kind == "q" else (s, idx * 512)

            prog = {"f": 0, "m": 0, "o": 0}

            def pump(done_sbank):
                while prog["f"] < NQT and NEED_SBANK[prog["f"]] <= done_sbank:
                    attn_front(prog["f"])
                    prog["f"] += 1
                    while prog["m"] < max(0, prog["f"] - 1):
                        attn_mid(prog["m"])
                        prog["m"] += 1
                    while prog["o"] < max(0, prog["m"] - 1):
                        attn_fin(prog["o"])
                        prog["o"] += 1

            # input bank 0 first on the DMA queue so LN starts immediately;
            # small constants next (needed early), then the rest behind.
            pending = [ln_load(*bank_src(0))]
            rqt_sb = consts.tile([128, 6], F32)
            nc.sync.dma_start(out=rqt_sb, in_=rqt[:, :])
            lns_sb = consts.tile([128, NT, 2], F32)
            nc.sync.dma_start(out=lns_sb, in_=lns[:, :, :])
            wqk_sb = wts.tile([128, 4, DIM + RANK], BF16, tag="wqk")
            nc.sync.dma_start(
                out=wqk_sb, in_=wqk[:, :].rearrange("(c p) n -> p c n", p=128))
            wkg_sb = wts.tile([128, 4, RANK], F8, tag="wkg")
            nc.sync.dma_start(
                out=wkg_sb, in_=wkg[:, :].rearrange("(c p) n -> p c n", p=128))
            pending.append(ln_load(*bank_src(1)))
            mskall = msks.tile([128, NQT, WINW], BF16, tag="msk")
            nc.sync.dma_start(
                out=mskall,
                in_=bmask[:, :, :].rearrange("t p n -> p t n"))
            msk_t = [mskall[:, t, :] for t in range(NQT)]
            wvo_sb = wts.tile([128, 4, DIM], BF16, tag="wvo")
            nc.sync.dma_start(
                out=wvo_sb, in_=wvo[:, :].rearrange("(c p) n -> p c n", p=128))
            bo2r_sb = consts.tile([128, DIM], F32)
            nc.sync.dma_start(out=bo2r_sb, in_=bo2r[:, :])

            for bk in range(10):
                kind, idx = BANKS[bk]
                xq = pending.pop(0)
                if bk + 2 < 10:
                    pending.append(ln_load(*bank_src(bk + 2)))
                col0 = idx * 512
                dst_big = qt_big if kind == "q" else st8
                for jj in range(2):
                    pt2 = ps_t.tile([128, 4, 256], BF16, tag="pt")
                    for j2 in range(2):
                        j = jj * 2 + j2
                        if kind == "q":
                            xn = xnp.tile([128, DIM], BF16, tag="xn")
                            g = idx * 4 + j
                        else:
                            xn = xnr[:, idx * 4 + j, :]
                            g = NQT + idx * 4 + j
                        ln_tile(xq[:, j, :], xn, pt2, j2 * 128, g)
                    if jj == 0:
                        nc.vector.tensor_copy(
                            dst_big[:, :, col0:col0 + 256], pt2)
                    else:
                        nc.scalar.activation(
                            out=dst_big[:, :, col0 + 256:col0 + 512],
                            in_=pt2, func=FT.Identity, bias=0.0, scale=1.0)
                if kind == "q":
                    proj_q_bank(col0)
                else:
                    proj_gk_bank(col0)
                    pump(idx)
            while prog["m"] < NQT:
                attn_mid(prog["m"])
                prog["m"] += 1
            while prog["o"] < NQT:
                attn_fin(prog["o"])
                prog["o"] += 1

    if not nc.is_finalized():
        nc.finalize()
    return nc


_NC_CACHE = None


def _get_nc():
    global _NC_CACHE
    if _NC_CACHE is None:
        _NC_CACHE = build_bass()
    return _NC_CACHE


def _host_fold(inputs):
    f32 = np.float32
    bf16 = mybir.dt.np(BF16)
    f8 = mybir.dt.np(F8)
    scale = f32(DIM ** -0.5)
    sqr = f32(np.sqrt(RANK))
    ctx0 = np.asarray(inputs["ctx0"], f32)
    ctx1 = np.asarray(inputs["ctx1"], f32)
    pre = ctx0 @ inputs["Wc0"] + inputs["bc0"] + ctx1 @ inputs["Wc1"] + inputs["bc1"]
    pre = np.asarray(pre, f32)
    h = pre / (1.0 + np.exp(-pre))
    gbv = np.asarray(h @ inputs["Wf"] + inputs["bf"], f32)
    gamma, beta = gbv[:, :DIM], gbv[:, DIM:]

    qn_g = np.asarray(inputs["qn_g"], f32)
    qn_b = np.asarray(inputs["qn_b"], f32)
    kvn_g = np.asarray(inputs["kvn_g"], f32)
    kvn_b = np.asarray(inputs["kvn_b"], f32)
    Wq, bq = np.asarray(inputs["Wq"], f32), np.asarray(inputs["bq"], f32)
    Wk, bk = np.asarray(inputs["Wk"], f32), np.asarray(inputs["bk"], f32)
    Wv, bv = np.asarray(inputs["Wv"], f32), np.asarray(inputs["bv"], f32)
    Wo, bo = np.asarray(inputs["Wo"], f32), np.asarray(inputs["bo"], f32)
    Wgq = np.asarray(inputs["Wgq"], f32)
    Wgk = np.asarray(inputs["Wgk"], f32)
    mask = np.asarray(inputs["mask"], f32)

    # batch-independent folds
    WkS = Wk * kvn_g[:, None]
    rk = (kvn_b @ Wk + bk).astype(f32)
    WvS = Wv * kvn_g[:, None]
    rv = (kvn_b @ Wv + bv).astype(f32)
    wvo = np.ascontiguousarray(WvS @ Wo).astype(bf16)          # [512, 512]
    bo2r = np.broadcast_to((rv @ Wo + bo).astype(f32), (128, DIM))
    bo2r = np.ascontiguousarray(bo2r)
    wkg8 = np.ascontiguousarray((WkS @ Wgk) * GK_PRESC).astype(f8)
    rkg = (rk @ Wgk).astype(f32)                               # true gate-k bias

    bmask = np.stack([mask[t * 128:(t + 1) * 128, w:w + WINW]
                      for t, w in enumerate(WSTARTS)])
    bmask = (np.maximum(bmask, -50.0) * QK_PRESC).astype(bf16)

    query = np.asarray(inputs["query"], f32).reshape(B, Q, DIM)
    source = np.asarray(inputs["source"], f32).reshape(B, K, DIM)

    in_maps = []
    for b in range(B):
        sg = qn_g * (1.0 + gamma[b])
        off = qn_b * (1.0 + gamma[b]) + beta[b]
        Wq_f = Wq * sg[:, None]
        rq_raw = (off @ Wq + bq).astype(f32)
        Wqk = (Wq_f @ WkS.T) * (scale * QK_PRESC)
        rqk = (rq_raw @ WkS.T) * (scale * QK_PRESC)
        wgq_f = (Wq_f @ Wgq) / sqr
        rgq = rq_raw @ Wgq / sqr
        wqk_ext = np.concatenate([Wqk, wgq_f], axis=1)
        rqt_b = np.zeros((128, 6), f32)
        rqt_b[:, :4] = rqk.reshape(4, 128).T
        rqt_b[:RANK, 4] = rgq
        rqt_b[:RANK, 5] = rkg

        # host LN stats: rstd and -mu*rstd per row, tile-major [128, NT, 2]
        xall = np.concatenate([query[b], source[b]], axis=0)   # [5120, 512]
        mu = xall.mean(axis=1)
        var = xall.var(axis=1)
        rstd = 1.0 / np.sqrt(var + 1e-5)
        nmr = -mu * rstd
        lns_b = np.stack([rstd.reshape(NT, 128).T,
                          nmr.reshape(NT, 128).T], axis=2).astype(f32)

        in_maps.append({
            "q": query[b].astype(bf16),
            "s": source[b].astype(bf16),
            "wqk": wqk_ext.astype(bf16),
            "wkg": wkg8,
            "wvo": wvo,
            "rqt": rqt_b,
            "lns": np.ascontiguousarray(lns_b),
            "bo2r": bo2r,
            "bmask": bmask,
        })
    return in_maps


def kernel(**inputs):
    nc = _get_nc()
    in_maps = _host_fold(inputs)
    res = run_bass_kernel_spmd(nc, in_maps, core_ids=list(range(B)))
    out = np.stack([res.results[b]["out"] for b in range(B)])
    return out.reshape(B, QS, QT, DIM).astype(np.float32)


if __name__ == "__main__":
    build_bass()
    print("bass build OK")


# revision 15
# speedup vs baseline: 1.3288x; 1.1857x over previous
"""Trainium2 Bass kernel for nn_LocalDenseCrossReadout (v5, folded projections).

Strategy:
- Data-parallel over batch: 8 batches -> 8 NeuronCores, one batch per core.
- Host-side algebraic folds:
  * k-projection eliminated: scores = q_p @ k_p^T = (q_p @ Wk^T) @ xn_s^T
    (+ per-q-row constant q_p.rk which cancels in softmax), so we fold
    Wqk = Wq_f @ Wk^T per batch and project only the 1024 q rows.
  * v-projection and output projection merged: attn @ (xn_s @ Wv) @ Wo =
    (attn @ xn_s) @ (Wv @ Wo): attention reads raw LN'd source rows and the
    output projection uses Wvo = Wv @ Wo; v bias folds into the output bias.
  * gate_q rides as extra columns of the q projection; gate_k =
    xn_s @ (Wk @ Wgk) is a small fp8 DoubleRow projection.
  * LayerNorm statistics (row mean/rstd) are computed on host in f32 and
    shipped as a [128, 40, 2] tensor; the device applies x*rstd + (-mu*rstd)
    on Pool. This keeps the Scalar activation table set to {exp, tanh,
    identity} (one table, no reload thrash).
- Device per core: LN apply + PE transposes (bf16 q / fp8 s column-major),
  bf16 q-projection, fp8 DoubleRow gate-k + scores, banded attention per
  128-row q tile (tanh gate, mask folded into score psum, P=(1+t)*e with
  fused row-sum), attn@xn via DMA-transposed P, output projection with
  normalize+bias fused into one DVE op.
"""

import sys

sys.path.insert(0, "/opt/trn_rl_repo")

import numpy as np

import concourse.bass as bass
import concourse.tile as tile
from concourse import bacc
from concourse import mybir
from concourse.bass_utils import run_bass_kernel_spmd
from concourse.masks import make_identity

DIM, QS, QT, KS, KT, WIN, B, RANK = 512, 64, 16, 256, 16, 4, 8, 32
Q = QS * QT  # 1024
K = KS * KT  # 4096
WINW = 768  # aligned kv window per 128-row q tile
NQT = Q // 128  # 8 q tiles
NKV = K // 128  # 32 kv tiles
NT = NQT + NKV  # 40 LN tiles
F32 = mybir.dt.float32
BF16 = mybir.dt.bfloat16
F8 = mybir.dt.float8e4
FT = mybir.ActivationFunctionType
ALU = mybir.AluOpType

QK_PRESC = 64.0   # q-proj psum = QK_PRESC * scale * qk ; exp descales
GK_PRESC = 256.0  # gate-k psum prescale (fp8 weight range)

# kv window start (aligned to 128) and width per q tile
WSTARTS = [0, 384, 896, 1408, 1920, 2432, 2944, 3456]
WINWS = [640, 768, 768, 768, 768, 768, 768, 640]
# last s-bank (512 rows each) needed per q tile
NEED_SBANK = [(w + ww + 511) // 512 - 1 for w, ww in zip(WSTARTS, WINWS)]


def build_bass():
    nc = bacc.Bacc("TRN2", target_bir_lowering=False)
    q = nc.dram_tensor("q", [Q, DIM], BF16, kind="ExternalInput")
    s = nc.dram_tensor("s", [K, DIM], BF16, kind="ExternalInput")
    wqk = nc.dram_tensor("wqk", [DIM, DIM + RANK], BF16, kind="ExternalInput")
    wkg = nc.dram_tensor("wkg", [DIM, RANK], F8, kind="ExternalInput")
    wvo = nc.dram_tensor("wvo", [DIM, DIM], BF16, kind="ExternalInput")
    rqt = nc.dram_tensor("rqt", [128, 6], F32, kind="ExternalInput")
    lns = nc.dram_tensor("lns", [128, NT, 2], F32, kind="ExternalInput")
    bo2r = nc.dram_tensor("bo2r", [128, DIM], F32, kind="ExternalInput")
    bmask = nc.dram_tensor("bmask", [NQT, 128, WINW], BF16, kind="ExternalInput")
    out = nc.dram_tensor("out", [Q, DIM], F32, kind="ExternalOutput")

    with tile.TileContext(nc) as tc:
        with (
            tc.tile_pool(name="consts", bufs=1) as consts,
            tc.tile_pool(name="wts", bufs=1) as wts,
            tc.tile_pool(name="big", bufs=1) as big,
            tc.tile_pool(name="xin", bufs=3) as xin,
            tc.tile_pool(name="xnp", bufs=4) as xnp,
            tc.tile_pool(name="stats", bufs=8) as stats,
            tc.tile_pool(name="attn", bufs=3) as attn,
            tc.tile_pool(name="msks", bufs=1) as msks,
            tc.tile_pool(name="ps_t", bufs=2, space="PSUM") as ps_t,
            tc.tile_pool(name="ps_p", bufs=2, space="PSUM") as ps_p,
            tc.tile_pool(name="ps_b", bufs=2, space="PSUM") as ps_b,
        ):
            # ---------------- constants ----------------
            identb = consts.tile([128, 128], BF16)
            make_identity(nc, identb)

            # persistent activations
            qt_big = big.tile([128, 4, Q], BF16, tag="qt_big")   # xn_q^T (bf16)
            st8 = big.tile([128, 4, K], F8, tag="st8")           # xn_s^T (fp8)
            xnr = big.tile([128, NKV, DIM], BF16, tag="xnr")     # xn_s rows
            qkT = big.tile([128, 4, Q], F8, tag="qkT")           # (qk*scale*64)^T
            gq = big.tile([32, Q], BF16, tag="gq")               # gate_q^T
            gk = big.tile([32, K], BF16, tag="gk")               # gate_k^T

            # ---- stage 1: load one bank (512 rows) in one DMA
            def ln_load(src_dram, row0):
                x4 = xin.tile([128, 4, DIM], BF16, tag="x")
                nc.sync.dma_start(
                    out=x4, in_=src_dram[row0:row0 + 512, :].rearrange(
                        "(j p) n -> p j n", p=128))
                return x4

            # ---- stage 2: LN apply (host stats) + PE transpose
            def ln_tile(x, xn_dst, pt2, joff, g):
                nc.gpsimd.tensor_scalar(out=xn_dst, in0=x,
                                        scalar1=lns_sb[:, g, 0:1],
                                        scalar2=lns_sb[:, g, 1:2],
                                        op0=ALU.mult, op1=ALU.add)
                for c in range(4):
                    nc.tensor.transpose(pt2[:, c, joff:joff + 128],
                                        xn_dst[:, c * 128:(c + 1) * 128], identb)

            # ---- q projection (qk + gate_q), both banks fused per m-block
            def proj_q_both():
                for m in range(5):
                    mw = 128 if m < 4 else RANK
                    mo = m * 128
                    pp0 = ps_p.tile([128, DIM], F32, tag="ps")
                    pp1 = ps_p.tile([128, DIM], F32, tag="ps")
                    pps = [pp0, pp1]
                    for c in range(4):
                        for h in range(2):
                            nc.tensor.matmul(
                                pps[h][:mw, :], wqk_sb[:, c, mo:mo + mw],
                                qt_big[:, c, h * 512:h * 512 + 512],
                                start=(c == 0), stop=(c == 3))
                    for h in range(2):
                        col0 = h * 512
                        if m < 4:
                            nc.scalar.activation(
                                out=qkT[:, m, col0:col0 + 512], in_=pps[h],
                                func=FT.Identity,
                                bias=rqt_sb[:, m:m + 1], scale=1.0)
                        else:
                            nc.scalar.activation(
                                out=gq[:, col0:col0 + 512], in_=pps[h][:RANK, :],
                                func=FT.Identity,
                                bias=rqt_sb[:RANK, 4:5], scale=1.0)

            # ---- gate_k projection (fp8 DoubleRow) for one s bank
            def proj_gk_bank(col0):
                pp = ps_p.tile([128, DIM], F32, tag="ps")
                for c in (0, 2):
                    nc.tensor.matmul(pp[:RANK, :], wkg_sb[:, c:c + 2, :],
                                     st8[:, c:c + 2, col0:col0 + 512],
                                     start=(c == 0), stop=(c == 2),
                                     perf_mode=mybir.MatmulPerfMode.DoubleRow)
                nc.scalar.activation(out=gk[:, col0:col0 + 512],
                                     in_=pp[:RANK, :], func=FT.Identity,
                                     bias=rqt_sb[:RANK, 5:6],
                                     scale=1.0 / GK_PRESC)

            state = {}

            def attn_front(t):
                """gate logits, tanh, scores+mask, exp, P, P^T."""
                w0, ww = WSTARTS[t], WINWS[t]
                splits = ((0, 512), (512, ww - 512))
                qc = bass.ts(t, 128)
                gl = ps_b.tile([128, WINW], F32, tag="glsc")
                for n0, nn_ in splits:
                    nc.tensor.matmul(gl[:, n0:n0 + nn_], gq[:, qc],
                                     gk[:, w0 + n0:w0 + n0 + nn_],
                                     start=True, stop=True)
                # 2*sigmoid(gl) = 1 + tanh(gl/2); factor 2 cancels in softmax
                tq = attn.tile([128, WINW], BF16, tag="tq")
                nc.scalar.activation(out=tq[:, :ww], in_=gl[:, :ww],
                                     func=FT.Tanh, bias=0.0, scale=0.5)
                sc = ps_b.tile([128, WINW], F32, tag="glsc")
                for c in (0, 2):
                    for n0, nn_ in splits:
                        nc.tensor.matmul(sc[:, n0:n0 + nn_],
                                         qkT[:, c:c + 2, qc],
                                         st8[:, c:c + 2, w0 + n0:w0 + n0 + nn_],
                                         start=(c == 0), stop=False,
                                         perf_mode=mybir.MatmulPerfMode.DoubleRow)
                for i, (n0, nn_) in enumerate(splits):
                    nc.tensor.matmul(sc[:, n0:n0 + nn_], identb,
                                     msk_t[t][:, n0:n0 + nn_],
                                     start=False, stop=True)
                e = attn.tile([128, WINW], BF16, tag="e")
                nc.scalar.activation(out=e[:, :ww], in_=sc[:, :ww], func=FT.Exp,
                                     bias=0.0, scale=1.0 / QK_PRESC)
                P = attn.tile([128, WINW], BF16, tag="P")
                rsum = stats.tile([128, 1], F32, tag="rsum")
                nc.vector.scalar_tensor_tensor(
                    out=P[:, :ww], in0=tq[:, :ww], scalar=1.0, in1=e[:, :ww],
                    op0=ALU.add, op1=ALU.mult, accum_out=rsum)
                rinv = stats.tile([128, 1], F32, tag="rinv")
                nc.vector.reciprocal(out=rinv, in_=rsum)
                aT = attn.tile([128, 6, 128], BF16, tag="aT")
                nc.sync.dma_start_transpose(aT[:, :ww // 128, :], P[:, :ww])
                state[t] = (aT, rinv)

            def attn_mid(t):
                """attn@xn (unnormalized), oa^T via DMA transpose."""
                w0, nch = WSTARTS[t], WINWS[t] // 128
                aT, rinv = state.pop(t)
                av = ps_p.tile([128, DIM], F32, tag="ps")
                for cc in range(nch):
                    nc.tensor.matmul(av, aT[:, cc, :], xnr[:, w0 // 128 + cc, :],
                                     start=(cc == 0), stop=(cc == nch - 1))
                oa = attn.tile([128, DIM], BF16, tag="oa")
                nc.vector.tensor_copy(oa, av)
                oaT = attn.tile([128, 4, 128], BF16, tag="oaT")
                nc.scalar.dma_start_transpose(oaT, oa)
                state[(t, "fin")] = (oaT, rinv)

            def attn_fin(t):
                """output projection, fused normalize + bias, store."""
                qc = bass.ts(t, 128)
                oaT, rinv = state.pop((t, "fin"))
                fin = ps_p.tile([128, DIM], F32, tag="ps")
                for c in range(4):
                    nc.tensor.matmul(fin, oaT[:, c, :], wvo_sb[:, c, :],
                                     start=(c == 0), stop=(c == 3))
                ob = attn.tile([128, DIM], F32, tag="ob")
                nc.vector.scalar_tensor_tensor(
                    out=ob, in0=fin, scalar=rinv, in1=bo2r_sb,
                    op0=ALU.mult, op1=ALU.add)
                nc.gpsimd.dma_start(out=out[qc, :], in_=ob)

            # ---- bank schedule: q0, s0, s1, q1, s2..s7 -----------------
            BANKS = [("q", 0), ("s", 0), ("s", 1), ("q", 1),
                     ("s", 2), ("s", 3), ("s", 4), ("s", 5), ("s", 6), ("s", 7)]

            def bank_src(bk):
                kind, idx = BANKS[bk]
                return (q, idx * 512) if kind == "q" else (s, idx * 512)

            prog = {"f": 0, "m": 0, "o": 0}

            def pump(done_sbank):
                while prog["f"] < NQT and NEED_SBANK[prog["f"]] <= done_sbank:
                    attn_front(prog["f"])
                    prog["f"] += 1
                    while prog["m"] < max(0, prog["f"] - 1):
                        attn_mid(prog["m"])
                        prog["m"] += 1
                    while prog["o"] < max(0, prog["m"] - 1):
                        attn_fin(prog["o"])
                        prog["o"] += 1

            # input bank 0 first on the DMA queue so LN starts immediately;
            # small constants next (needed early), then the rest behind.
            pending = [ln_load(*bank_src(0))]
            rqt_sb = consts.tile([128, 6], F32)
            nc.sync.dma_start(out=rqt_sb, in_=rqt[:, :])
            lns_sb = consts.tile([128, NT, 2], F32)
            nc.sync.dma_start(out=lns_sb, in_=lns[:, :, :])
            wqk_sb = wts.tile([128, 4, DIM + RANK], BF16, tag="wqk")
            nc.sync.dma_start(
                out=wqk_sb, in_=wqk[:, :].rearrange("(c p) n -> p c n", p=128))
            wkg_sb = wts.tile([128, 4, RANK], F8, tag="wkg")
            nc.sync.dma_start(
                out=wkg_sb, in_=wkg[:, :].rearrange("(c p) n -> p c n", p=128))
            pending.append(ln_load(*bank_src(1)))
            mskall = msks.tile([128, NQT, WINW], BF16, tag="msk")
            nc.sync.dma_start(
                out=mskall,
                in_=bmask[:, :, :].rearrange("t p n -> p t n"))
            msk_t = [mskall[:, t, :] for t in range(NQT)]
            wvo_sb = wts.tile([128, 4, DIM], BF16, tag="wvo")
            nc.sync.dma_start(
                out=wvo_sb, in_=wvo[:, :].rearrange("(c p) n -> p c n", p=128))
            bo2r_sb = consts.tile([128, DIM], F32)
            nc.sync.dma_start(out=bo2r_sb, in_=bo2r[:, :])

            sdone = -1
            for bk in range(10):
                kind, idx = BANKS[bk]
                xq = pending.pop(0)
                if bk + 2 < 10:
                    pending.append(ln_load(*bank_src(bk + 2)))
                col0 = idx * 512
                dst_big = qt_big if kind == "q" else st8
                for jj in range(2):
                    pt2 = ps_t.tile([128, 4, 256], BF16, tag="pt")
                    for j2 in range(2):
                        j = jj * 2 + j2
                        if kind == "q":
                            xn = xnp.tile([128, DIM], BF16, tag="xn")
                            g = idx * 4 + j
                        else:
                            xn = xnr[:, idx * 4 + j, :]
                            g = NQT + idx * 4 + j
                        ln_tile(xq[:, j, :], xn, pt2, j2 * 128, g)
                    if jj == 0:
                        nc.vector.tensor_copy(
                            dst_big[:, :, col0:col0 + 256], pt2)
                    else:
                        nc.scalar.activation(
                            out=dst_big[:, :, col0 + 256:col0 + 512],
                            in_=pt2, func=FT.Identity, bias=0.0, scale=1.0)
                if kind == "q":
                    if idx == 1:
                        proj_q_both()
                        pump(sdone)
                else:
                    proj_gk_bank(col0)
                    sdone = idx
                    if bk >= 3:
                        pump(idx)
            while prog["m"] < NQT:
                attn_mid(prog["m"])
                prog["m"] += 1
            while prog["o"] < NQT:
                attn_fin(prog["o"])
                prog["o"] += 1

    if not nc.is_finalized():
        nc.finalize()
    return nc


_NC_CACHE = None


def _get_nc():
    global _NC_CACHE
    if _NC_CACHE is None:
        _NC_CACHE = build_bass()
    return _NC_CACHE


def _host_fold(inputs):
    f32 = np.float32
    bf16 = mybir.dt.np(BF16)
    f8 = mybir.dt.np(F8)
    scale = f32(DIM ** -0.5)
    sqr = f32(np.sqrt(RANK))
    ctx0 = np.asarray(inputs["ctx0"], f32)
    ctx1 = np.asarray(inputs["ctx1"], f32)
    pre = ctx0 @ inputs["Wc0"] + inputs["bc0"] + ctx1 @ inputs["Wc1"] + inputs["bc1"]
    pre = np.asarray(pre, f32)
    h = pre / (1.0 + np.exp(-pre))
    gbv = np.asarray(h @ inputs["Wf"] + inputs["bf"], f32)
    gamma, beta = gbv[:, :DIM], gbv[:, DIM:]

    qn_g = np.asarray(inputs["qn_g"], f32)
    qn_b = np.asarray(inputs["qn_b"], f32)
    kvn_g = np.asarray(inputs["kvn_g"], f32)
    kvn_b = np.asarray(inputs["kvn_b"], f32)
    Wq, bq = np.asarray(inputs["Wq"], f32), np.asarray(inputs["bq"], f32)
    Wk, bk = np.asarray(inputs["Wk"], f32), np.asarray(inputs["bk"], f32)
    Wv, bv = np.asarray(inputs["Wv"], f32), np.asarray(inputs["bv"], f32)
    Wo, bo = np.asarray(inputs["Wo"], f32), np.asarray(inputs["bo"], f32)
    Wgq = np.asarray(inputs["Wgq"], f32)
    Wgk = np.asarray(inputs["Wgk"], f32)
    mask = np.asarray(inputs["mask"], f32)

    # batch-independent folds
    WkS = Wk * kvn_g[:, None]
    rk = (kvn_b @ Wk + bk).astype(f32)
    WvS = Wv * kvn_g[:, None]
    rv = (kvn_b @ Wv + bv).astype(f32)
    wvo = np.ascontiguousarray(WvS @ Wo).astype(bf16)          # [512, 512]
    bo2r = np.broadcast_to((rv @ Wo + bo).astype(f32), (128, DIM))
    bo2r = np.ascontiguousarray(bo2r)
    wkg8 = np.ascontiguousarray((WkS @ Wgk) * GK_PRESC).astype(f8)
    rkg = (rk @ Wgk).astype(f32)                               # true gate-k bias

    bm = np.full((NQT, 128, WINW), -50.0, np.float32)
    for t, (w, ww) in enumerate(zip(WSTARTS, WINWS)):
        bm[t, :, :ww] = np.maximum(mask[t * 128:(t + 1) * 128, w:w + ww], -50.0)
    bmask = (bm * QK_PRESC).astype(bf16)

    query = np.asarray(inputs["query"], f32).reshape(B, Q, DIM)
    source = np.asarray(inputs["source"], f32).reshape(B, K, DIM)

    in_maps = []
    for b in range(B):
        sg = qn_g * (1.0 + gamma[b])
        off = qn_b * (1.0 + gamma[b]) + beta[b]
        Wq_f = Wq * sg[:, None]
        rq_raw = (off @ Wq + bq).astype(f32)
        Wqk = (Wq_f @ WkS.T) * (scale * QK_PRESC)
        rqk = (rq_raw @ WkS.T) * (scale * QK_PRESC)
        wgq_f = (Wq_f @ Wgq) / sqr
        rgq = rq_raw @ Wgq / sqr
        wqk_ext = np.concatenate([Wqk, wgq_f], axis=1)
        rqt_b = np.zeros((128, 6), f32)
        rqt_b[:, :4] = rqk.reshape(4, 128).T
        rqt_b[:RANK, 4] = rgq
        rqt_b[:RANK, 5] = rkg

        # host LN stats: rstd and -mu*rstd per row, tile-major [128, NT, 2]
        xall = np.concatenate([query[b], source[b]], axis=0)   # [5120, 512]
        mu = xall.mean(axis=1)
        var = xall.var(axis=1)
        rstd = 1.0 / np.sqrt(var + 1e-5)
        nmr = -mu * rstd
        lns_b = np.stack([rstd.reshape(NT, 128).T,
                          nmr.reshape(NT, 128).T], axis=2).astype(f32)

        in_maps.append({
            "q": query[b].astype(bf16),
            "s": source[b].astype(bf16),
            "wqk": wqk_ext.astype(bf16),
            "wkg": wkg8,
            "wvo": wvo,
            "rqt": rqt_b,
            "lns": np.ascontiguousarray(lns_b),
            "bo2r": bo2r,
            "bmask": bmask,
        })
    return in_maps


def kernel(**inputs):
    nc = _get_nc()
    in_maps = _host_fold(inputs)
    res = run_bass_kernel_spmd(nc, in_maps, core_ids=list(range(B)))
    out = np.stack([res.results[b]["out"] for b in range(B)])
    return out.reshape(B, QS, QT, DIM).astype(np.float32)


if __name__ == "__main__":
    build_bass()
    print("bass build OK")


# revision 16
# speedup vs baseline: 1.3289x; 1.0001x over previous
"""Trainium2 Bass kernel for nn_LocalDenseCrossReadout (v5, folded projections).

Strategy:
- Data-parallel over batch: 8 batches -> 8 NeuronCores, one batch per core.
- Host-side algebraic folds:
  * k-projection eliminated: scores = q_p @ k_p^T = (q_p @ Wk^T) @ xn_s^T
    (+ per-q-row constant q_p.rk which cancels in softmax), so we fold
    Wqk = Wq_f @ Wk^T per batch and project only the 1024 q rows.
  * v-projection and output projection merged: attn @ (xn_s @ Wv) @ Wo =
    (attn @ xn_s) @ (Wv @ Wo): attention reads raw LN'd source rows and the
    output projection uses Wvo = Wv @ Wo; v bias folds into the output bias.
  * gate_q rides as extra columns of the q projection; gate_k =
    xn_s @ (Wk @ Wgk) is a small fp8 DoubleRow projection.
  * LayerNorm statistics (row mean/rstd) are computed on host in f32 and
    shipped as a [128, 40, 2] tensor; the device applies x*rstd + (-mu*rstd)
    on Pool. This keeps the Scalar activation table set to {exp, tanh,
    identity} (one table, no reload thrash).
- Device per core: LN apply + PE transposes (bf16 q / fp8 s column-major),
  bf16 q-projection, fp8 DoubleRow gate-k + scores, banded attention per
  128-row q tile (tanh gate, mask folded into score psum, P=(1+t)*e with
  fused row-sum), attn@xn via DMA-transposed P, output projection with
  normalize+bias fused into one DVE op.
"""

import sys

sys.path.insert(0, "/opt/trn_rl_repo")

import numpy as np

import concourse.bass as bass
import concourse.tile as tile
from concourse import bacc
from concourse import mybir
from concourse.bass_utils import run_bass_kernel_spmd
from concourse.masks import make_identity

DIM, QS, QT, KS, KT, WIN, B, RANK = 512, 64, 16, 256, 16, 4, 8, 32
Q = QS * QT  # 1024
K = KS * KT  # 4096
WINW = 768  # aligned kv window per 128-row q tile
NQT = Q // 128  # 8 q tiles
NKV = K // 128  # 32 kv tiles
NT = NQT + NKV  # 40 LN tiles
F32 = mybir.dt.float32
BF16 = mybir.dt.bfloat16
F8 = mybir.dt.float8e4
FT = mybir.ActivationFunctionType
ALU = mybir.AluOpType

QK_PRESC = 64.0   # q-proj psum = QK_PRESC * scale * qk ; exp descales
GK_PRESC = 256.0  # gate-k psum prescale (fp8 weight range)

# kv window start (aligned to 128) and width per q tile
WSTARTS = [0, 384, 896, 1408, 1920, 2432, 2944, 3456]
WINWS = [640, 768, 768, 768, 768, 768, 768, 640]
# last s-bank (512 rows each) needed per q tile
NEED_SBANK = [(w + ww + 511) // 512 - 1 for w, ww in zip(WSTARTS, WINWS)]


def build_bass():
    nc = bacc.Bacc("TRN2", target_bir_lowering=False)
    q = nc.dram_tensor("q", [Q, DIM], BF16, kind="ExternalInput")
    s = nc.dram_tensor("s", [K, DIM], BF16, kind="ExternalInput")
    wqk = nc.dram_tensor("wqk", [DIM, DIM + RANK], BF16, kind="ExternalInput")
    wkg = nc.dram_tensor("wkg", [DIM, RANK], F8, kind="ExternalInput")
    wvo = nc.dram_tensor("wvo", [DIM, DIM], BF16, kind="ExternalInput")
    rqt = nc.dram_tensor("rqt", [128, 6], F32, kind="ExternalInput")
    lns = nc.dram_tensor("lns", [128, NT, 2], F32, kind="ExternalInput")
    bo2r = nc.dram_tensor("bo2r", [128, DIM], F32, kind="ExternalInput")
    bmask = nc.dram_tensor("bmask", [NQT, 128, WINW], BF16, kind="ExternalInput")
    out = nc.dram_tensor("out", [Q, DIM], F32, kind="ExternalOutput")

    with tile.TileContext(nc) as tc:
        with (
            tc.tile_pool(name="consts", bufs=1) as consts,
            tc.tile_pool(name="wts", bufs=1) as wts,
            tc.tile_pool(name="big", bufs=1) as big,
            tc.tile_pool(name="xin", bufs=3) as xin,
            tc.tile_pool(name="xnp", bufs=4) as xnp,
            tc.tile_pool(name="stats", bufs=8) as stats,
            tc.tile_pool(name="attn", bufs=3) as attn,
            tc.tile_pool(name="msks", bufs=1) as msks,
            tc.tile_pool(name="ps_t", bufs=2, space="PSUM") as ps_t,
            tc.tile_pool(name="ps_p", bufs=2, space="PSUM") as ps_p,
            tc.tile_pool(name="ps_b", bufs=2, space="PSUM") as ps_b,
        ):
            # ---------------- constants ----------------
            identb = consts.tile([128, 128], BF16)
            make_identity(nc, identb)

            # persistent activations
            qt_big = big.tile([128, 4, Q], BF16, tag="qt_big")   # xn_q^T (bf16)
            st8 = big.tile([128, 4, K], F8, tag="st8")           # xn_s^T (fp8)
            xnr = big.tile([128, NKV, DIM], BF16, tag="xnr")     # xn_s rows
            qkT = big.tile([128, 4, Q], F8, tag="qkT")           # (qk*scale*64)^T
            gq = big.tile([32, Q], BF16, tag="gq")               # gate_q^T
            gk = big.tile([32, K], BF16, tag="gk")               # gate_k^T

            # ---- stage 1: load one bank (512 rows); split halves for bank 0
            def ln_load(src_dram, row0, split=False):
                x4 = xin.tile([128, 4, DIM], BF16, tag="x")
                if split:
                    for hh in range(2):
                        nc.sync.dma_start(
                            out=x4[:, hh * 2:hh * 2 + 2, :],
                            in_=src_dram[row0 + hh * 256:row0 + hh * 256 + 256,
                                         :].rearrange("(j p) n -> p j n", p=128))
                else:
                    nc.sync.dma_start(
                        out=x4, in_=src_dram[row0:row0 + 512, :].rearrange(
                            "(j p) n -> p j n", p=128))
                return x4

            # ---- stage 2: LN apply (host stats) + PE transpose
            def ln_tile(x, xn_dst, pt2, joff, g):
                nc.gpsimd.tensor_scalar(out=xn_dst, in0=x,
                                        scalar1=lns_sb[:, g, 0:1],
                                        scalar2=lns_sb[:, g, 1:2],
                                        op0=ALU.mult, op1=ALU.add)
                for c in range(4):
                    nc.tensor.transpose(pt2[:, c, joff:joff + 128],
                                        xn_dst[:, c * 128:(c + 1) * 128], identb)

            # ---- q projection (qk + gate_q), both banks fused per m-block
            def proj_q_both():
                for m in range(5):
                    mw = 128 if m < 4 else RANK
                    mo = m * 128
                    pp0 = ps_p.tile([128, DIM], F32, tag="ps")
                    pp1 = ps_p.tile([128, DIM], F32, tag="ps")
                    pps = [pp0, pp1]
                    for c in range(4):
                        for h in range(2):
                            nc.tensor.matmul(
                                pps[h][:mw, :], wqk_sb[:, c, mo:mo + mw],
                                qt_big[:, c, h * 512:h * 512 + 512],
                                start=(c == 0), stop=(c == 3))
                    for h in range(2):
                        col0 = h * 512
                        if m < 4:
                            nc.scalar.activation(
                                out=qkT[:, m, col0:col0 + 512], in_=pps[h],
                                func=FT.Identity,
                                bias=rqt_sb[:, m:m + 1], scale=1.0)
                        else:
                            nc.scalar.activation(
                                out=gq[:, col0:col0 + 512], in_=pps[h][:RANK, :],
                                func=FT.Identity,
                                bias=rqt_sb[:RANK, 4:5], scale=1.0)

            # ---- gate_k projection (fp8 DoubleRow) for one s bank
            def proj_gk_bank(col0):
                pp = ps_p.tile([128, DIM], F32, tag="ps")
                for c in (0, 2):
                    nc.tensor.matmul(pp[:RANK, :], wkg_sb[:, c:c + 2, :],
                                     st8[:, c:c + 2, col0:col0 + 512],
                                     start=(c == 0), stop=(c == 2),
                                     perf_mode=mybir.MatmulPerfMode.DoubleRow)
                nc.vector.tensor_scalar(out=gk[:, col0:col0 + 512],
                                        in0=pp[:RANK, :],
                                        scalar1=1.0 / GK_PRESC,
                                        scalar2=rqt_sb[:RANK, 5:6],
                                        op0=ALU.mult, op1=ALU.add)

            state = {}

            def attn_front(t):
                """gate logits, tanh, scores+mask, exp, P, P^T."""
                w0, ww = WSTARTS[t], WINWS[t]
                splits = ((0, 512), (512, ww - 512))
                qc = bass.ts(t, 128)
                gl = ps_b.tile([128, WINW], F32, tag="glsc")
                for n0, nn_ in splits:
                    nc.tensor.matmul(gl[:, n0:n0 + nn_], gq[:, qc],
                                     gk[:, w0 + n0:w0 + n0 + nn_],
                                     start=True, stop=True)
                # 2*sigmoid(gl) = 1 + tanh(gl/2); factor 2 cancels in softmax
                tq = attn.tile([128, WINW], BF16, tag="tq")
                nc.scalar.activation(out=tq[:, :ww], in_=gl[:, :ww],
                                     func=FT.Tanh, bias=0.0, scale=0.5)
                sc = ps_b.tile([128, WINW], F32, tag="glsc")
                for c in (0, 2):
                    for n0, nn_ in splits:
                        nc.tensor.matmul(sc[:, n0:n0 + nn_],
                                         qkT[:, c:c + 2, qc],
                                         st8[:, c:c + 2, w0 + n0:w0 + n0 + nn_],
                                         start=(c == 0), stop=False,
                                         perf_mode=mybir.MatmulPerfMode.DoubleRow)
                for i, (n0, nn_) in enumerate(splits):
                    nc.tensor.matmul(sc[:, n0:n0 + nn_], identb,
                                     msk_t[t][:, n0:n0 + nn_],
                                     start=False, stop=True)
                e = attn.tile([128, WINW], BF16, tag="e")
                nc.scalar.activation(out=e[:, :ww], in_=sc[:, :ww], func=FT.Exp,
                                     bias=0.0, scale=1.0 / QK_PRESC)
                P = attn.tile([128, WINW], BF16, tag="P")
                rsum = stats.tile([128, 1], F32, tag="rsum")
                nc.vector.scalar_tensor_tensor(
                    out=P[:, :ww], in0=tq[:, :ww], scalar=1.0, in1=e[:, :ww],
                    op0=ALU.add, op1=ALU.mult, accum_out=rsum)
                rinv = stats.tile([128, 1], F32, tag="rinv")
                nc.vector.reciprocal(out=rinv, in_=rsum)
                aT = attn.tile([128, 6, 128], BF16, tag="aT")
                nc.sync.dma_start_transpose(aT[:, :ww // 128, :], P[:, :ww])
                state[t] = (aT, rinv)

            def attn_mid(t):
                """attn@xn (unnormalized), oa^T via DMA transpose."""
                w0, nch = WSTARTS[t], WINWS[t] // 128
                aT, rinv = state.pop(t)
                av = ps_p.tile([128, DIM], F32, tag="ps")
                for cc in range(nch):
                    nc.tensor.matmul(av, aT[:, cc, :], xnr[:, w0 // 128 + cc, :],
                                     start=(cc == 0), stop=(cc == nch - 1))
                oa = attn.tile([128, DIM], BF16, tag="oa")
                nc.vector.tensor_copy(oa, av)
                oaT = attn.tile([128, 4, 128], BF16, tag="oaT")
                nc.sync.dma_start_transpose(oaT, oa)
                state[(t, "fin")] = (oaT, rinv)

            def attn_fin(t):
                """output projection, fused normalize + bias, store."""
                qc = bass.ts(t, 128)
                oaT, rinv = state.pop((t, "fin"))
                fin = ps_p.tile([128, DIM], F32, tag="ps")
                for c in range(4):
                    nc.tensor.matmul(fin, oaT[:, c, :], wvo_sb[:, c, :],
                                     start=(c == 0), stop=(c == 3))
                ob = attn.tile([128, DIM], F32, tag="ob")
                nc.vector.scalar_tensor_tensor(
                    out=ob, in0=fin, scalar=rinv, in1=bo2r_sb,
                    op0=ALU.mult, op1=ALU.add)
                nc.sync.dma_start(out=out[qc, :], in_=ob)

            # ---- bank schedule: q0, s0, s1, q1, s2..s7 -----------------
            BANKS = [("q", 0), ("s", 0), ("s", 1), ("q", 1),
                     ("s", 2), ("s", 3), ("s", 4), ("s", 5), ("s", 6), ("s", 7)]

            def bank_src(bk):
                kind, idx = BANKS[bk]
                return (q, idx * 512) if kind == "q" else (s, idx * 512)

            prog = {"f": 0, "m": 0, "o": 0}

            def pump(done_sbank):
                while prog["f"] < NQT and NEED_SBANK[prog["f"]] <= done_sbank:
                    attn_front(prog["f"])
                    prog["f"] += 1
                    while prog["m"] < max(0, prog["f"] - 1):
                        attn_mid(prog["m"])
                        prog["m"] += 1
                    while prog["o"] < max(0, prog["m"] - 1):
                        attn_fin(prog["o"])
                        prog["o"] += 1

            # input bank 0 first on the DMA queue so LN starts immediately;
            # small constants next (needed early), then the rest behind.
            pending = [ln_load(*bank_src(0), split=True)]
            rqt_sb = consts.tile([128, 6], F32)
            nc.sync.dma_start(out=rqt_sb, in_=rqt[:, :])
            lns_sb = consts.tile([128, NT, 2], F32)
            nc.sync.dma_start(out=lns_sb, in_=lns[:, :, :])
            wqk_sb = wts.tile([128, 4, DIM + RANK], BF16, tag="wqk")
            nc.sync.dma_start(
                out=wqk_sb, in_=wqk[:, :].rearrange("(c p) n -> p c n", p=128))
            wkg_sb = wts.tile([128, 4, RANK], F8, tag="wkg")
            nc.sync.dma_start(
                out=wkg_sb, in_=wkg[:, :].rearrange("(c p) n -> p c n", p=128))
            pending.append(ln_load(*bank_src(1)))
            mskall = msks.tile([128, NQT, WINW], BF16, tag="msk")
            nc.sync.dma_start(
                out=mskall,
                in_=bmask[:, :, :].rearrange("t p n -> p t n"))
            msk_t = [mskall[:, t, :] for t in range(NQT)]
            wvo_sb = wts.tile([128, 4, DIM], BF16, tag="wvo")
            nc.sync.dma_start(
                out=wvo_sb, in_=wvo[:, :].rearrange("(c p) n -> p c n", p=128))
            bo2r_sb = consts.tile([128, DIM], F32)
            nc.sync.dma_start(out=bo2r_sb, in_=bo2r[:, :])

            sdone = -1
            for bk in range(10):
                kind, idx = BANKS[bk]
                xq = pending.pop(0)
                if bk + 2 < 10:
                    pending.append(ln_load(*bank_src(bk + 2)))
                col0 = idx * 512
                dst_big = qt_big if kind == "q" else st8
                for jj in range(2):
                    pt2 = ps_t.tile([128, 4, 256], BF16, tag="pt")
                    for j2 in range(2):
                        j = jj * 2 + j2
                        if kind == "q":
                            xn = xnp.tile([128, DIM], BF16, tag="xn")
                            g = idx * 4 + j
                        else:
                            xn = xnr[:, idx * 4 + j, :]
                            g = NQT + idx * 4 + j
                        ln_tile(xq[:, j, :], xn, pt2, j2 * 128, g)
                    if jj == 0:
                        nc.vector.tensor_copy(
                            dst_big[:, :, col0:col0 + 256], pt2)
                    else:
                        nc.scalar.activation(
                            out=dst_big[:, :, col0 + 256:col0 + 512],
                            in_=pt2, func=FT.Identity, bias=0.0, scale=1.0)
                if kind == "q":
                    if idx == 1:
                        proj_q_both()
                        pump(sdone)
                else:
                    proj_gk_bank(col0)
                    sdone = idx
                    if bk >= 3:
                        pump(idx)
            while prog["m"] < NQT:
                attn_mid(prog["m"])
                prog["m"] += 1
            while prog["o"] < NQT:
                attn_fin(prog["o"])
                prog["o"] += 1

    if not nc.is_finalized():
        nc.finalize()
    return nc


_NC_CACHE = None


def _get_nc():
    global _NC_CACHE
    if _NC_CACHE is None:
        _NC_CACHE = build_bass()
    return _NC_CACHE


def _host_fold(inputs):
    f32 = np.float32
    bf16 = mybir.dt.np(BF16)
    f8 = mybir.dt.np(F8)
    scale = f32(DIM ** -0.5)
    sqr = f32(np.sqrt(RANK))
    ctx0 = np.asarray(inputs["ctx0"], f32)
    ctx1 = np.asarray(inputs["ctx1"], f32)
    pre = ctx0 @ inputs["Wc0"] + inputs["bc0"] + ctx1 @ inputs["Wc1"] + inputs["bc1"]
    pre = np.asarray(pre, f32)
    h = pre / (1.0 + np.exp(-pre))
    gbv = np.asarray(h @ inputs["Wf"] + inputs["bf"], f32)
    gamma, beta = gbv[:, :DIM], gbv[:, DIM:]

    qn_g = np.asarray(inputs["qn_g"], f32)
    qn_b = np.asarray(inputs["qn_b"], f32)
    kvn_g = np.asarray(inputs["kvn_g"], f32)
    kvn_b = np.asarray(inputs["kvn_b"], f32)
    Wq, bq = np.asarray(inputs["Wq"], f32), np.asarray(inputs["bq"], f32)
    Wk, bk = np.asarray(inputs["Wk"], f32), np.asarray(inputs["bk"], f32)
    Wv, bv = np.asarray(inputs["Wv"], f32), np.asarray(inputs["bv"], f32)
    Wo, bo = np.asarray(inputs["Wo"], f32), np.asarray(inputs["bo"], f32)
    Wgq = np.asarray(inputs["Wgq"], f32)
    Wgk = np.asarray(inputs["Wgk"], f32)
    mask = np.asarray(inputs["mask"], f32)

    # batch-independent folds
    WkS = Wk * kvn_g[:, None]
    rk = (kvn_b @ Wk + bk).astype(f32)
    WvS = Wv * kvn_g[:, None]
    rv = (kvn_b @ Wv + bv).astype(f32)
    wvo = np.ascontiguousarray(WvS @ Wo).astype(bf16)          # [512, 512]
    bo2r = np.broadcast_to((rv @ Wo + bo).astype(f32), (128, DIM))
    bo2r = np.ascontiguousarray(bo2r)
    wkg8 = np.ascontiguousarray((WkS @ Wgk) * GK_PRESC).astype(f8)
    rkg = (rk @ Wgk).astype(f32)                               # true gate-k bias

    bm = np.full((NQT, 128, WINW), -50.0, np.float32)
    for t, (w, ww) in enumerate(zip(WSTARTS, WINWS)):
        bm[t, :, :ww] = np.maximum(mask[t * 128:(t + 1) * 128, w:w + ww], -50.0)
    bmask = (bm * QK_PRESC).astype(bf16)

    query = np.asarray(inputs["query"], f32).reshape(B, Q, DIM)
    source = np.asarray(inputs["source"], f32).reshape(B, K, DIM)

    in_maps = []
    for b in range(B):
        sg = qn_g * (1.0 + gamma[b])
        off = qn_b * (1.0 + gamma[b]) + beta[b]
        Wq_f = Wq * sg[:, None]
        rq_raw = (off @ Wq + bq).astype(f32)
        Wqk = (Wq_f @ WkS.T) * (scale * QK_PRESC)
        rqk = (rq_raw @ WkS.T) * (scale * QK_PRESC)
        wgq_f = (Wq_f @ Wgq) / sqr
        rgq = rq_raw @ Wgq / sqr
        wqk_ext = np.concatenate([Wqk, wgq_f], axis=1)
        rqt_b = np.zeros((128, 6), f32)
        rqt_b[:, :4] = rqk.reshape(4, 128).T
        rqt_b[:RANK, 4] = rgq
        rqt_b[:RANK, 5] = rkg

        # host LN stats: rstd and -mu*rstd per row, tile-major [128, NT, 2]
        xall = np.concatenate([query[b], source[b]], axis=0)   # [5120, 512]
        mu = xall.mean(axis=1)
        var = xall.var(axis=1)
        rstd = 1.0 / np.sqrt(var + 1e-5)
        nmr = -mu * rstd
        lns_b = np.stack([rstd.reshape(NT, 128).T,
                          nmr.reshape(NT, 128).T], axis=2).astype(f32)

        in_maps.append({
            "q": query[b].astype(bf16),
            "s": source[b].astype(bf16),
            "wqk": wqk_ext.astype(bf16),
            "wkg": wkg8,
            "wvo": wvo,
            "rqt": rqt_b,
            "lns": np.ascontiguousarray(lns_b),
            "bo2r": bo2r,
            "bmask": bmask,
        })
    return in_maps


def kernel(**inputs):
    nc = _get_nc()
    in_maps = _host_fold(inputs)
    res = run_bass_kernel_spmd(nc, in_maps, core_ids=list(range(B)))
    out = np.stack([res.results[b]["out"] for b in range(B)])
    return out.reshape(B, QS, QT, DIM).astype(np.float32)


if __name__ == "__main__":
    build_bass()
    print("bass build OK")
